# revision 1
# baseline (speedup 1.0000x reference)
"""Causal self-attention (B=4, S=2048, D=2048, H=16) on 8 TRN2 NeuronCores.

Sharding: core c -> batch b=c//2, tensor-parallel half t=c%2 (8 heads each).
Each core computes QKV projections for its 8 heads, causal attention, and a
partial out-projection; the host sums the two TP partials per batch.

All heavy matmuls run in float32r (E8M11, 4x faster than fp32 on the PE,
~1.5e-4 relative rounding), accumulating in fp32 PSUM. Inputs are pre-rounded
to the f32r grid on the host, so no on-chip dtype conversions are needed.

Projections run in two sequence halves (x^T half resident at a time); since
causal attention on chunk c consumes only projections of chunks <= c, the
attention pass over the first half is emitted (and scheduled) between the two
projection halves. Phases communicate via per-(head, chunk) DRAM scratch;
attention context is spilled to DRAM and re-read by the out-projection, whose
weights prefetch into the space vacated by x^T.
"""
import math
from contextlib import ExitStack

import numpy as np

import concourse.bass as bass
import concourse.bacc as bacc
import concourse.mybir as mybir
import concourse.tile as tile
from concourse.bass_utils import run_bass_kernel_spmd

B, S, D, H, HD = 4, 2048, 2048, 16, 128
HL = 8              # heads per core
ML = HL * HD        # local model dims (1024)
P = 128
NDT = D // P        # 16 contraction tiles
NST = S // P        # 16 seq tiles
NSC = S // 512      # 4 seq chunks
ISQ = 1.0 / math.sqrt(HD)
F32 = mybir.dt.float32
F32R = mybir.dt.float32r
Exp = mybir.ActivationFunctionType.Exp

_BUILT = {}


def _build():
    nc = bacc.Bacc("TRN2", target_bir_lowering=False, debug=False, num_devices=8)
    xT = nc.declare_dram_parameter("xT", [D, S], F32R, isOutput=False)
    wqT = nc.declare_dram_parameter("wqT", [D, ML], F32R, isOutput=False)
    wkT = nc.declare_dram_parameter("wkT", [D, ML], F32R, isOutput=False)
    wvT = nc.declare_dram_parameter("wvT", [D, ML], F32R, isOutput=False)
    woT = nc.declare_dram_parameter("woT", [ML, D], F32R, isOutput=False)
    mask0 = nc.declare_dram_parameter("mask0", [P, P], F32, isOutput=False)
    out = nc.declare_dram_parameter("out", [S, D], F32, isOutput=True)

    with tile.TileContext(nc) as tc, ExitStack() as top:
        dp = top.enter_context(tc.tile_pool(name="dram", bufs=1, space="DRAM"))
        # fine-grained scratch: one DRAM tile per (head, chunk) / (mq, half)
        sq = [[dp.tile([P, 512], F32R, tag=f"sq{h}_{c}", name=f"sq{h}_{c}")
               for c in range(NSC)] for h in range(HL)]
        sk = [[dp.tile([P, 512], F32R, tag=f"sk{h}_{c}", name=f"sk{h}_{c}")
               for c in range(NSC)] for h in range(HL)]
        sv = [[dp.tile([1024, 256], F32R, tag=f"sv{m}_{g}", name=f"sv{m}_{g}")
               for g in range(2)] for m in range(4)]
        sctx = [dp.tile([P, S], F32R, tag=f"sctx{h}", name=f"sctx{h}")
                for h in range(HL)]

        xT_r = xT.ap().rearrange("(t p) s -> p t s", p=P)
        wqT_r = wqT.ap().rearrange("(t p) m -> p t m", p=P)
        wkT_r = wkT.ap().rearrange("(t p) m -> p t m", p=P)
        wvT_r = wvT.ap().rearrange("(t p) m -> p t m", p=P)

        # phase-1 (left side) and phase-2 (right side) pools coexist
        p1 = ExitStack()
        xr_pool = p1.enter_context(
            tc.tile_pool(name="xr_pool", bufs=1, side="left"))
        vw = p1.enter_context(tc.tile_pool(name="vw", bufs=2, side="left"))
        qkw = p1.enter_context(tc.tile_pool(name="qkw", bufs=2, side="left"))
        p1ev = p1.enter_context(tc.tile_pool(name="p1ev", bufs=3, side="left"))
        p1ps = p1.enter_context(
            tc.tile_pool(name="p1ps", bufs=2, space="PSUM", side="left"))

        p2 = ExitStack()
        const = p2.enter_context(
            tc.tile_pool(name="const", bufs=1, side="right"))
        qk2 = p2.enter_context(tc.tile_pool(name="qk2", bufs=2, side="right"))
        p2w = p2.enter_context(tc.tile_pool(name="p2w", bufs=1, side="right"))
        p2ps = p2.enter_context(
            tc.tile_pool(name="p2ps", bufs=2, space="PSUM", side="right"))
        p2ps1 = p2.enter_context(
            tc.tile_pool(name="p2ps1", bufs=1, space="PSUM", side="right"))

        # attention constants, loaded up front
        m0 = const.tile([P, P], F32, tag="m0")
        nc.sync.dma_start(out=m0[:], in_=mask0[:])
        ones_f = const.tile([P, 1], F32, tag="ones_f")
        nc.vector.memset(ones_f[:], 1.0)
        ones_col = const.tile([P, 1], F32R, tag="ones_col")
        nc.vector.tensor_copy(ones_col[:], ones_f[:])
        onesr_f = const.tile([1, P], F32, tag="onesr_f")
        nc.vector.memset(onesr_f[:], 1.0)
        ones_row = const.tile([1, P], F32R, tag="ones_row")
        nc.vector.tensor_copy(ones_row[:], onesr_f[:])

        def attention_chunk(h, c, k_g, v_g, koff):
            """Emit attention for (head h, i-chunk c). k_g covers k chunks
            [0, koff) of the head; v_g covers the matching s-tiles."""
            nj = 4 * c + 4
            ndiag = 4 * c  # J >= ndiag are diagonal tiles
            q_c = qk2.tile([P, 512], F32R, tag="q_c", bufs=3, name=f"q_{h}_{c}")
            nc.sync.dma_start(out=q_c[:], in_=sq[h][c][:])
            pctx = p2ps.tile([P, 512], F32, tag="pctx", name=f"pctx_{h}_{c}")
            pden = p2ps1.tile([1, 512], F32, tag="pden", name=f"pden_{h}_{c}")
            # denominator reduction: non-diagonal pt tiles are pre-summed in
            # quads on the DVE so the PE streams them once per quad
            n_pden = (ndiag + 3) // 4 + (nj - ndiag)
            pden_idx = 0
            ptsum = None
            quad = 0
            for J in range(nj):
                diag = J >= ndiag
                r = J * P - c * 512 if diag else 0
                pscr = p2ps.tile([P, 512], F32, tag="pscr",
                                 name=f"pscr_{h}_{c}_{J}")
                nc.tensor.matmul(
                    pscr[:, r:512],
                    k_g[:, J * P:(J + 1) * P], q_c[:, r:512],
                    start=True, stop=True)
                pt = p2w.tile([P, 512], F32R, tag="pt", bufs=7,
                              name=f"pt_{h}_{c}_{J}")
                nc.scalar.activation(pt[:, r:512], pscr[:, r:512], Exp,
                                     scale=ISQ)
                if diag:
                    nc.vector.tensor_mul(
                        pt[:, r:r + P], pt[:, r:r + P], m0[:])
                nc.tensor.matmul(
                    pctx[:, r:512], v_g[:, J, :], pt[:, r:512],
                    start=(J == 0), stop=(J == nj - 1))
                if diag:
                    nc.tensor.matmul(
                        pden[:, r:512], ones_col[:], pt[:, r:512],
                        start=(pden_idx == 0), stop=(pden_idx == n_pden - 1))
                    pden_idx += 1
                else:
                    if quad == 0:
                        ptsum = pt
                    else:
                        ptsum2 = p2w.tile([P, 512], F32R, tag="ptsum",
                                          bufs=2, name=f"pts_{h}_{c}_{J}")
                        nc.vector.tensor_add(ptsum2[:], ptsum[:], pt[:])
                        ptsum = ptsum2
                    quad += 1
                    if quad == 4 or J == ndiag - 1:
                        nc.tensor.matmul(
                            pden[:], ones_col[:], ptsum[:],
                            start=(pden_idx == 0),
                            stop=(pden_idx == n_pden - 1))
                        pden_idx += 1
                        quad = 0
                        ptsum = None
            # 1/denominator, broadcast to 128 partitions via K=1 matmul
            recip = p2w.tile([1, 512], F32, tag="recip", name=f"rc_{h}_{c}")
            nc.vector.reciprocal(recip[:], pden[:])
            recip_r = p2w.tile([1, 512], F32R, tag="recip_r",
                               name=f"rcr_{h}_{c}")
            nc.vector.tensor_copy(recip_r[:], recip[:])
            pbc = p2ps1.tile([P, 512], F32, tag="pbc", name=f"pbc_{h}_{c}")
            nc.tensor.matmul(pbc[:], ones_row[:], recip_r[:],
                             start=True, stop=True)
            rb = p2w.tile([P, 512], F32, tag="rb", name=f"rb_{h}_{c}")
            nc.vector.tensor_copy(rb[:], pbc[:])
            cv = p2w.tile([P, 512], F32R, tag="cv", bufs=2, name=f"cv_{h}_{c}")
            nc.vector.tensor_mul(cv[:], pctx[:], rb[:])
            nc.sync.dma_start(
                out=sctx[h][:, c * 512:(c + 1) * 512], in_=cv[:])

        for g in range(2):
            # ---- Phase 1 half g: x^T half resident [128, 16, 1024] ----
            xr = xr_pool.tile([P, NDT, 1024], F32R, tag="xr", name=f"xr{g}")
            if g == 0:
                # fine-grained first loads so the first v matmul group (which
                # needs only s-tile 0 and the first Wv quarter) starts early
                for q4 in range(4):
                    nc.sync.dma_start(
                        out=xr[:, :, q4 * P:(q4 + 1) * P],
                        in_=xT_r[:, :, q4 * P:(q4 + 1) * P])
                nc.sync.dma_start(out=xr[:, :, 512:1024],
                                  in_=xT_r[:, :, 512:1024])
            else:
                for lc in range(2):
                    nc.sync.dma_start(
                        out=xr[:, :, lc * 512:(lc + 1) * 512],
                        in_=xT_r[:, :, (2 * g + lc) * 512:
                                 (2 * g + lc + 1) * 512])

            # v: [s-tile 128, m-quarter 256]; stationary = x, moving = Wv
            for mq in range(4):
                vr = vw.tile([P, NDT, 256], F32R, tag="vr", name=f"vr{g}_{mq}")
                nc.sync.dma_start(
                    out=vr[:], in_=wvT_r[:, :, mq * 256:(mq + 1) * 256])
                for stl in range(8):
                    ps = p1ps.tile([P, 512], F32, tag="pp")
                    for d in range(NDT):
                        nc.tensor.matmul(
                            ps[:, 0:256], xr[:, d, stl * P:(stl + 1) * P],
                            vr[:, d, :],
                            start=(d == 0), stop=(d == NDT - 1))
                    ev = p1ev.tile([P, 512], F32R, tag="ev")
                    nc.scalar.copy(ev[:, 0:256], ps[:, 0:256])
                    nc.sync.dma_start(
                        out=sv[mq][g][stl * P:(stl + 1) * P, :],
                        in_=ev[:, 0:256])

            # q^T / k^T per head: [m-tile 128, s-chunk]
            for h in range(HL):
                for wsrc, dst in ((wqT_r, sq[h]), (wkT_r, sk[h])):
                    wr = qkw.tile([P, NDT, P], F32R, tag="wr")
                    nc.sync.dma_start(
                        out=wr[:], in_=wsrc[:, :, h * P:(h + 1) * P])
                    for lc in range(2):
                        ps = p1ps.tile([P, 512], F32, tag="pp")
                        for d in range(NDT):
                            nc.tensor.matmul(
                                ps[:], wr[:, d, :],
                                xr[:, d, lc * 512:(lc + 1) * 512],
                                start=(d == 0), stop=(d == NDT - 1))
                        ev = p1ev.tile([P, 512], F32R, tag="ev")
                        nc.scalar.copy(ev[:], ps[:])
                        nc.sync.dma_start(out=dst[2 * g + lc][:], in_=ev[:])

            # ---- Attention pass g: chunks 2g, 2g+1 for every head ----
            koff = 2 * (g + 1)  # k/v chunks needed: [0, koff)
            for h in range(HL):
                k_g = qk2.tile([P, koff * 512], F32R, tag=f"k{g}",
                               name=f"k{g}_{h}")
                for c in range(koff):
                    nc.sync.dma_start(
                        out=k_g[:, c * 512:(c + 1) * 512], in_=sk[h][c][:])
                v_g = qk2.tile([P, koff * 4, P], F32R, tag=f"v{g}",
                               name=f"v{g}_{h}")
                for gg in range(g + 1):
                    nc.sync.dma_start(
                        out=v_g[:, gg * 8:(gg + 1) * 8, :],
                        in_=sv[h // 2][gg][:].rearrange(
                            "(t p) m -> p t m", p=P)[
                            :, :, (h % 2) * P:(h % 2) * P + P])
                for lc in range(2):
                    attention_chunk(h, 2 * g + lc, k_g, v_g, koff)

        # free phase-1 space (x^T etc.); wo prefetch reuses it
        p1.close()

        # ---------------- Phase 3: out-projection ----------------
        with tc.tile_pool(name="wo_pool", bufs=1, side="left") as wo_pool, \
             tc.tile_pool(name="p3sb", bufs=3, side="left") as p3sb, \
             tc.tile_pool(name="p3ev", bufs=3, side="left") as p3ev, \
             tc.tile_pool(name="p3ps", bufs=2, space="PSUM",
                          side="left") as p3ps:
            woT_r = woT.ap().rearrange("(h p) n -> p h n", p=P)
            wo = wo_pool.tile([P, HL, D], F32R, tag="wo")
            for h in range(HL):
                nc.sync.dma_start(out=wo[:, h, :], in_=woT_r[:, h, :])
            for st in range(NST):
                cx = p3sb.tile([P, HL, P], F32R, tag="cx")
                for h in range(HL):
                    nc.sync.dma_start(
                        out=cx[:, h, :],
                        in_=sctx[h][:, st * P:(st + 1) * P])
                for nk in range(NSC):
                    ps = p3ps.tile([P, 512], F32, tag="po")
                    for h in range(HL):
                        nc.tensor.matmul(
                            ps[:], cx[:, h, :],
                            wo[:, h, nk * 512:(nk + 1) * 512],
                            start=(h == 0), stop=(h == HL - 1))
                    ev = p3ev.tile([P, 512], F32, tag="evo")
                    nc.scalar.copy(ev[:], ps[:])
                    nc.sync.dma_start(
                        out=out[st * P:(st + 1) * P,
                                nk * 512:(nk + 1) * 512], in_=ev[:])

        p2.close()

    nc.finalize()
    return nc


def get_nc():
    if "nc" not in _BUILT:
        _BUILT["nc"] = _build()
    return _BUILT["nc"]


def _to_f32r(a):
    """Round fp32 to the float32r (E8M11) grid: RNE at 12 low mantissa bits."""
    u = np.ascontiguousarray(a, dtype=np.float32).view(np.uint32)
    r = (u + 0x7FF + ((u >> 12) & 1)) & np.uint32(0xFFFFF000)
    return r.view(np.float32)


def _make_in_maps(x, Wq, Wk, Wv, Wo):
    jj, ff = np.meshgrid(np.arange(P), np.arange(P), indexing="ij")
    mask0 = (ff >= jj).astype(np.float32)
    in_maps = []
    for c in range(8):
        b, t = c // 2, c % 2
        ms = slice(t * ML, (t + 1) * ML)
        in_maps.append({
            "xT": _to_f32r(x[b].T),
            "wqT": _to_f32r(Wq[ms, :].T),
            "wkT": _to_f32r(Wk[ms, :].T),
            "wvT": _to_f32r(Wv[ms, :].T),
            "woT": _to_f32r(Wo[:, ms].T),
            "mask0": mask0,
        })
    return in_maps


def kernel(x, Wq, Wk, Wv, Wo):
    x = np.asarray(x, dtype=np.float32)
    Wq = np.asarray(Wq, dtype=np.float32)
    Wk = np.asarray(Wk, dtype=np.float32)
    Wv = np.asarray(Wv, dtype=np.float32)
    Wo = np.asarray(Wo, dtype=np.float32)

    nc = get_nc()
    in_maps = _make_in_maps(x, Wq, Wk, Wv, Wo)
    res = run_bass_kernel_spmd(nc, in_maps, list(range(8)))
    outs = [res.results[c]["out"] for c in range(8)]
    full = np.stack([outs[2 * b] + outs[2 * b + 1] for b in range(B)])
    return full.astype(np.float32)



# revision 18
# speedup vs baseline: 1.2078x; 1.2078x over previous
"""Causal self-attention (B=4, S=2048, D=2048, H=16) on 8 TRN2 NeuronCores.

Sharding: core c -> batch b=c//2, tensor-parallel half t=c%2 (8 heads each).
Each core computes QKV projections for its 8 heads, causal attention, and a
partial out-projection; the host sums the two TP partials per batch.

All on-chip data is bf16 (same PE rate as f32r on TRN2, half the DMA/SBUF),
accumulating in fp32 PSUM. q/k/v/ctx stay SBUF-resident for the whole run --
no DRAM scratch round-trips. The sequence is processed in four 512-token
quarters; the PE instruction stream for cycle c interleaves, at matmul
granularity, attention over quarter c with the QKV projections of quarter
c+1 and the out-projection of quarter c-1, so the Activation engine's exp
latency never stalls the PE. The softmax denominator is built on the Pool
engine (running tile sums + a partition all-reduce), costing zero PE work.
"""
import math
from contextlib import ExitStack

import numpy as np

import concourse.bass as bass
import concourse.bacc as bacc
import concourse.bass_isa as bass_isa
import concourse.mybir as mybir
import concourse.tile as tile
from concourse.bass_utils import run_bass_kernel_spmd

B, S, D, H, HD = 4, 2048, 2048, 16, 128
HL = 8              # heads per core
ML = HL * HD        # local model dims (1024)
P = 128
NDT = D // P        # 16 contraction tiles
NQ = 4              # sequence quarters
QS = S // NQ        # 512 tokens per quarter
ISQ = 1.0 / math.sqrt(HD)
F32 = mybir.dt.float32
BF16 = mybir.dt.bfloat16
Exp = mybir.ActivationFunctionType.Exp

_BUILT = {}


def _build():
    nc = bacc.Bacc("TRN2", target_bir_lowering=False, debug=False, num_devices=8)
    xT = nc.declare_dram_parameter("xT", [D, S], BF16, isOutput=False)
    # per head h: cols [256h, 256h+128) = Wq head cols, [256h+128, 256h+256) = Wk
    wqkT = nc.declare_dram_parameter("wqkT", [D, 2 * ML], BF16, isOutput=False)
    wvT = nc.declare_dram_parameter("wvT", [D, ML], BF16, isOutput=False)
    woT = nc.declare_dram_parameter("woT", [ML, D], BF16, isOutput=False)
    mask0 = nc.declare_dram_parameter("mask0", [P, P], BF16, isOutput=False)
    out = nc.declare_dram_parameter("out", [S, D], BF16, isOutput=True)

    xT_r = xT.ap().rearrange("(t p) s -> p t s", p=P)
    wqkT_r = wqkT.ap().rearrange("(t p) m -> p t m", p=P)
    wvT_r = wvT.ap().rearrange("(t p) m -> p t m", p=P)
    woT_r = woT.ap().rearrange("(h p) n -> p h n", p=P)

    with tile.TileContext(nc) as tc, ExitStack() as top:
        const = top.enter_context(tc.tile_pool(name="const", bufs=1))
        xp = top.enter_context(tc.tile_pool(name="xp", bufs=2))
        wvp = top.enter_context(tc.tile_pool(name="wvp", bufs=1))
        wqkp = top.enter_context(tc.tile_pool(name="wqkp", bufs=2))
        wop = top.enter_context(tc.tile_pool(name="wop", bufs=1))
        kvp = top.enter_context(tc.tile_pool(name="kvp", bufs=1))
        qp = top.enter_context(tc.tile_pool(name="qp", bufs=2))
        ctxp = top.enter_context(tc.tile_pool(name="ctxp", bufs=2))
        ptp = top.enter_context(tc.tile_pool(name="ptp", bufs=5))
        csp = top.enter_context(tc.tile_pool(name="csp", bufs=2))
        denp = top.enter_context(tc.tile_pool(name="denp", bufs=2))
        evp = top.enter_context(tc.tile_pool(name="evp", bufs=2))
        # PSUM: psA = double-width score tiles (2 banks x 2 bufs), psB = attn
        # context accumulators, psG = shared projection/out-projection groups
        psA = top.enter_context(tc.tile_pool(name="psA", bufs=2, space="PSUM"))
        psB = top.enter_context(tc.tile_pool(name="psB", bufs=2, space="PSUM"))
        psG = top.enter_context(tc.tile_pool(name="psG", bufs=2, space="PSUM"))

        m0 = const.tile([P, P], BF16, tag="m0", name="m0")

        # persistent per-run SBUF state
        k_tiles = [[kvp.tile([P, QS], BF16, tag=f"k{h}_{c}", name=f"k{h}_{c}")
                    for c in range(NQ)] for h in range(HL)]
        v_tiles = [kvp.tile([P, ML], BF16, tag=f"v{st}", name=f"v{st}")
                   for st in range(NQ * 4)]
        q_map = [[None] * HL for _ in range(NQ)]
        ctx_map = [[None] * HL for _ in range(NQ)]

        wo_strips = [None] * 4

        def load_wo():
            for nk in range(4):
                wo = wop.tile([P, HL, QS], BF16, tag=f"wo{nk}", name=f"wo{nk}")
                nc.sync.dma_start(
                    out=wo[:], in_=woT_r[:, :, nk * QS:(nk + 1) * QS])
                wo_strips[nk] = wo

        def outproj_gen(c, chunk):
            """Out-projection rows for the 4 s-tiles of quarter c."""
            cnt = 0
            for nk in range(4):
                wo = wo_strips[nk]
                for sti in range(4):
                    st = 4 * c + sti
                    ps = psG.tile([P, QS], F32, tag="pg", name=f"po{c}{nk}{sti}")
                    for h in range(HL):
                        nc.tensor.matmul(
                            ps[:],
                            ctx_map[c][h][:, sti * P:(sti + 1) * P],
                            wo[:, h, :],
                            start=(h == 0), stop=(h == HL - 1))
                        cnt += 1
                        if cnt % chunk == 0:
                            yield
                    ev = evp.tile([P, QS], BF16, tag="ev", name=f"ev{c}{nk}{sti}")
                    nc.scalar.copy(ev[:], ps[:])
                    nc.sync.dma_start(
                        out=out[st * P:(st + 1) * P, nk * QS:(nk + 1) * QS],
                        in_=ev[:])

        def attn_emit(h, c, slot):
            """Causal attention for (head h, query quarter c). `slot` is
            called ~once per k-tile to splice in interleaved matmuls.

            Off-diagonal k-tiles are processed in pairs sharing one
            double-width PSUM tile and a single exp over [128, 1024] (halves
            the Activation engine's fixed overhead); the P@V matmuls of each
            unit are emitted one unit late so the exp latency is hidden
            behind the next unit's score matmuls."""
            nj = 4 * (c + 1)
            ndiag = 4 * c
            qt = q_map[c][h]
            pctx = psB.tile([P, QS], F32, tag="pctx", name=f"pctx{h}_{c}")
            csum = csp.tile([P, QS], BF16, tag="csum", name=f"cs{h}_{c}")

            units = [("pair", J) for J in range(0, ndiag, 2)]
            units += [("diag", J) for J in range(ndiag, nj)]
            n_pv = 0
            pend = []

            def kslice(J):
                return k_tiles[h][J // 4][:, (J % 4) * P:(J % 4 + 1) * P]

            def flush():
                nonlocal n_pv
                for (pt_ap, J, r) in pend:
                    nc.tensor.matmul(
                        pctx[:, r:QS], v_tiles[J][:, h * P:(h + 1) * P],
                        pt_ap,
                        start=(n_pv == 0), stop=(n_pv == nj - 1))
                    n_pv += 1
                    if n_pv == 1:
                        nc.vector.tensor_copy(csum[:], pt_ap)
                    else:
                        nc.vector.tensor_add(
                            csum[:, r:QS], csum[:, r:QS], pt_ap)
                pend.clear()

            for kind, J in units:
                ps2 = psA.tile([P, 2 * QS], F32, tag="ps2",
                               name=f"sc{h}{c}{J}")
                if kind == "pair":
                    nc.tensor.matmul(ps2[:, 0:QS], kslice(J), qt[:],
                                     start=True, stop=True)
                    nc.tensor.matmul(ps2[:, QS:2 * QS], kslice(J + 1), qt[:],
                                     start=True, stop=True)
                    pt2 = ptp.tile([P, 2 * QS], BF16, tag="pt2", bufs=2,
                                   name=f"pt{h}{c}{J}")
                    nc.scalar.activation(pt2[:], ps2[:], Exp, scale=ISQ)
                    slot()
                    flush()
                    slot()
                    pend.append((pt2[:, 0:QS], J, 0))
                    pend.append((pt2[:, QS:2 * QS], J + 1, 0))
                else:
                    r = J * P - c * QS
                    nc.tensor.matmul(ps2[:, r:QS], kslice(J), qt[:, r:QS],
                                     start=True, stop=True)
                    pt = ptp.tile([P, QS], BF16, tag="pt", bufs=3,
                                  name=f"pt{h}{c}{J}")
                    nc.scalar.activation(pt[:, r:QS], ps2[:, r:QS], Exp,
                                         scale=ISQ)
                    nc.vector.tensor_mul(pt[:, r:r + P], pt[:, r:r + P], m0[:])
                    slot()
                    flush()
                    pend.append((pt[:, r:QS], J, r))
            flush()
            den = denp.tile([P, QS], BF16, tag="den", name=f"den{h}_{c}")
            nc.gpsimd.partition_all_reduce(
                den[:], csum[:], P, bass_isa.ReduceOp.add)
            rec = denp.tile([P, QS], BF16, tag="rec", name=f"rec{h}_{c}")
            with nc.allow_low_precision(reason="bf16 1/denominator, ~0.4% rel"):
                nc.vector.reciprocal(rec[:], den[:])
            ctx_t = ctxp.tile([P, QS], BF16, tag=f"ctx{h}", name=f"ctx{h}_{c}")
            nc.vector.tensor_mul(ctx_t[:], pctx[:], rec[:])
            ctx_map[c][h] = ctx_t

        def run_all(gen):
            for _ in gen:
                pass

        def make_slot(gens):
            """Round-robin one step from each live generator per call."""
            live = list(gens)

            def slot():
                for g in list(live):
                    try:
                        next(g)
                    except StopIteration:
                        live.remove(g)
            return slot, live

        def proj_gen2(c, chunk):
            """QKV projections for quarter c; yields after every `chunk`
            matmuls so the caller can interleave them into the PE stream.
            q/k for heads 0-3 go between the two Wv halves so the
            single-buffered vr reload overlaps the qk matmuls."""
            cnt = 0
            xq = xp.tile([P, NDT, QS], BF16, tag="xq", name=f"xq{c}")
            for i in range(4):
                nc.sync.dma_start(
                    out=xq[:, 4 * i:4 * i + 4, :],
                    in_=xT_r[:, 4 * i:4 * i + 4, c * QS:(c + 1) * QS])
            for mh in range(2):
                vr = wvp.tile([P, NDT, QS], BF16, tag="vr", name=f"vr{c}_{mh}")
                for i in range(4):
                    nc.sync.dma_start(
                        out=vr[:, 4 * i:4 * i + 4, :],
                        in_=wvT_r[:, 4 * i:4 * i + 4, mh * QS:(mh + 1) * QS])
                for sti in range(4):
                    st = 4 * c + sti
                    ps = psG.tile([P, QS], F32, tag="pg", name=f"pv{c}{mh}{sti}")
                    for d in range(NDT):
                        nc.tensor.matmul(
                            ps[:], xq[:, d, sti * P:(sti + 1) * P], vr[:, d, :],
                            start=(d == 0), stop=(d == NDT - 1))
                        cnt += 1
                        if cnt % chunk == 0:
                            yield
                    nc.vector.tensor_copy(
                        v_tiles[st][:, mh * QS:(mh + 1) * QS], ps[:])
                heads = range(4) if mh == 0 else range(4, HL)
                for h in heads:
                    wqk = wqkp.tile([P, NDT, 2 * P], BF16, tag="wqk",
                                    name=f"wqk{c}_{h}")
                    for i in range(2):
                        nc.sync.dma_start(
                            out=wqk[:, 8 * i:8 * i + 8, :],
                            in_=wqkT_r[:, 8 * i:8 * i + 8,
                                       h * 2 * P:(h + 1) * 2 * P])
                    qt = qp.tile([P, QS], BF16, tag=f"q{h}", name=f"q{h}_{c}")
                    q_map[c][h] = qt
                    for (off, dst) in ((0, qt), (P, k_tiles[h][c])):
                        ps = psG.tile([P, QS], F32, tag="pg",
                                      name=f"pq{c}{h}{off}")
                        for d in range(NDT):
                            nc.tensor.matmul(
                                ps[:], wqk[:, d, off:off + P], xq[:, d, :],
                                start=(d == 0), stop=(d == NDT - 1))
                            cnt += 1
                            if cnt % chunk == 0:
                                yield
                        nc.vector.tensor_copy(dst[:], ps[:])

        def prologue():
            """Quarter-0 projections with DMA piece interleave and, for the
            V part, 4 concurrently-open PSUM groups iterated d-piece-outer,
            so the PE starts as soon as the first 0.5MB pieces land and the
            DMA stream stays ahead of the 4x-slower PE consumption."""
            PIECES = [(0, 2), (2, 4), (4, 8), (8, 12), (12, 16)]
            xq = xp.tile([P, NDT, QS], BF16, tag="xq", name="xq0")
            vr0 = wvp.tile([P, NDT, QS], BF16, tag="vr", name="vr0_0")
            for (d0, d1) in PIECES:
                nc.sync.dma_start(out=xq[:, d0:d1, :],
                                  in_=xT_r[:, d0:d1, 0:QS])
                nc.sync.dma_start(out=vr0[:, d0:d1, :],
                                  in_=wvT_r[:, d0:d1, 0:QS])

            def v_half(mh, vr):
                psa = psA.tile([P, 2 * QS], F32, tag="ps2", name=f"pv0{mh}ab")
                ps = [psG.tile([P, QS], F32, tag="pg", name=f"pv0{mh}0"),
                      psG.tile([P, QS], F32, tag="pg", name=f"pv0{mh}1"),
                      psa[:, 0:QS], psa[:, QS:2 * QS]]
                for (d0, d1) in PIECES:
                    for g in range(4):
                        for d in range(d0, d1):
                            nc.tensor.matmul(
                                ps[g][:], xq[:, d, g * P:(g + 1) * P],
                                vr[:, d, :],
                                start=(d == 0), stop=(d == NDT - 1))
                for g in range(4):
                    nc.vector.tensor_copy(
                        v_tiles[g][:, mh * QS:(mh + 1) * QS], ps[g][:])

            def qk_heads(heads):
                for h in heads:
                    wqk = wqkp.tile([P, NDT, 2 * P], BF16, tag="wqk",
                                    name=f"wqk0_{h}")
                    for i in range(2):
                        nc.sync.dma_start(
                            out=wqk[:, 8 * i:8 * i + 8, :],
                            in_=wqkT_r[:, 8 * i:8 * i + 8,
                                       h * 2 * P:(h + 1) * 2 * P])
                    qt = qp.tile([P, QS], BF16, tag=f"q{h}", name=f"q{h}_0")
                    q_map[0][h] = qt
                    for (off, dst) in ((0, qt), (P, k_tiles[h][0])):
                        ps = psG.tile([P, QS], F32, tag="pg",
                                      name=f"pq0{h}{off}")
                        for d in range(NDT):
                            nc.tensor.matmul(
                                ps[:], wqk[:, d, off:off + P], xq[:, d, :],
                                start=(d == 0), stop=(d == NDT - 1))
                        nc.vector.tensor_copy(dst[:], ps[:])

            v_half(0, vr0)
            qk_heads(range(4))
            load_wo()
            nc.sync.dma_start(out=m0[:], in_=mask0[:])
            vr1 = wvp.tile([P, NDT, QS], BF16, tag="vr", name="vr0_1")
            for i in range(4):
                nc.sync.dma_start(out=vr1[:, 4 * i:4 * i + 4, :],
                                  in_=wvT_r[:, 4 * i:4 * i + 4, QS:2 * QS])
            v_half(1, vr1)
            qk_heads(range(4, HL))

        prologue()

        # chunk sizes per cycle chosen so each stream exhausts just before
        # its cycle's attention slots do (slots = 8 * nj(c))
        proj_chunk = {0: 12, 1: 6, 2: 4}
        op_chunk = {1: 2, 2: 2, 3: 1}
        for c in range(NQ):
            gens = []
            if c < 3:
                gens.append(proj_gen2(c + 1, proj_chunk[c]))
            if c >= 1:
                gens.append(outproj_gen(c - 1, op_chunk[c]))
            slot, live = make_slot(gens)
            for h in range(HL):
                attn_emit(h, c, slot)
            for g in live:
                run_all(g)
        run_all(outproj_gen(3, 10 ** 9))

    nc.finalize()
    return nc


def get_nc():
    if "nc" not in _BUILT:
        _BUILT["nc"] = _build()
    return _BUILT["nc"]


def _bf16(a):
    import ml_dtypes
    return np.ascontiguousarray(a, dtype=np.float32).astype(ml_dtypes.bfloat16)


def _make_in_maps(x, Wq, Wk, Wv, Wo):
    jj, ff = np.meshgrid(np.arange(P), np.arange(P), indexing="ij")
    mask0 = (ff >= jj).astype(np.float32)
    in_maps = []
    for c in range(8):
        b, t = c // 2, c % 2
        ms = slice(t * ML, (t + 1) * ML)
        wq = Wq[ms, :].T  # [D, ML]
        wk = Wk[ms, :].T
        # interleave per head: [q_h (128 cols) | k_h (128 cols)] blocks
        wqk = np.empty((D, 2 * ML), dtype=np.float32)
        for h in range(HL):
            wqk[:, 2 * h * P:(2 * h + 1) * P] = wq[:, h * P:(h + 1) * P]
            wqk[:, (2 * h + 1) * P:(2 * h + 2) * P] = wk[:, h * P:(h + 1) * P]
        in_maps.append({
            "xT": _bf16(x[b].T),
            "wqkT": _bf16(wqk),
            "wvT": _bf16(Wv[ms, :].T),
            "woT": _bf16(Wo[:, ms].T),
            "mask0": _bf16(mask0),
        })
    return in_maps


def kernel(x, Wq, Wk, Wv, Wo):
    x = np.asarray(x, dtype=np.float32)
    Wq = np.asarray(Wq, dtype=np.float32)
    Wk = np.asarray(Wk, dtype=np.float32)
    Wv = np.asarray(Wv, dtype=np.float32)
    Wo = np.asarray(Wo, dtype=np.float32)

    nc = get_nc()
    in_maps = _make_in_maps(x, Wq, Wk, Wv, Wo)
    res = run_bass_kernel_spmd(nc, in_maps, list(range(8)))
    outs = [np.asarray(res.results[c]["out"]).astype(np.float32)
            for c in range(8)]
    full = np.stack([outs[2 * b] + outs[2 * b + 1] for b in range(B)])
    return full.astype(np.float32)


# revision 23
# speedup vs baseline: 1.2278x; 1.0165x over previous
"""Causal self-attention (B=4, S=2048, D=2048, H=16) on 8 TRN2 NeuronCores.

Sharding: core c -> batch b=c//2, tensor-parallel half t=c%2 (8 heads each).
Each core computes QKV projections for its 8 heads, causal attention, and a
partial out-projection; the host sums the two TP partials per batch.

All on-chip data is bf16 (same PE rate as f32r on TRN2, half the DMA/SBUF),
accumulating in fp32 PSUM. q/k/v/ctx stay SBUF-resident for the whole run --
no DRAM scratch round-trips. The sequence is processed in four 512-token
quarters; the PE instruction stream for cycle c interleaves, at matmul
granularity, attention over quarter c with the QKV projections of quarter
c+1 and the out-projection of quarter c-1, so the Activation engine's exp
latency never stalls the PE. The softmax denominator is built on the Pool
engine (running tile sums + a partition all-reduce), costing zero PE work.
"""
import math
from contextlib import ExitStack

import numpy as np

import concourse.bass as bass
import concourse.bacc as bacc
import concourse.bass_isa as bass_isa
import concourse.mybir as mybir
import concourse.tile as tile
from concourse.bass_utils import run_bass_kernel_spmd

B, S, D, H, HD = 4, 2048, 2048, 16, 128
HL = 8              # heads per core
ML = HL * HD        # local model dims (1024)
P = 128
NDT = D // P        # 16 contraction tiles
NQ = 4              # sequence quarters
QS = S // NQ        # 512 tokens per quarter
ISQ = 1.0 / math.sqrt(HD)
F32 = mybir.dt.float32
BF16 = mybir.dt.bfloat16
Exp = mybir.ActivationFunctionType.Exp

_BUILT = {}


def _build():
    nc = bacc.Bacc("TRN2", target_bir_lowering=False, debug=False, num_devices=8)
    xT = nc.declare_dram_parameter("xT", [D, S], BF16, isOutput=False)
    # per head h: cols [256h, 256h+128) = Wq head cols, [256h+128, 256h+256) = Wk
    wqkT = nc.declare_dram_parameter("wqkT", [D, 2 * ML], BF16, isOutput=False)
    wvT = nc.declare_dram_parameter("wvT", [D, ML], BF16, isOutput=False)
    woT = nc.declare_dram_parameter("woT", [ML, D], BF16, isOutput=False)
    mask0 = nc.declare_dram_parameter("mask0", [P, P], BF16, isOutput=False)
    out = nc.declare_dram_parameter("out", [S, D], BF16, isOutput=True)

    xT_r = xT.ap().rearrange("(t p) s -> p t s", p=P)
    wqkT_r = wqkT.ap().rearrange("(t p) m -> p t m", p=P)
    wvT_r = wvT.ap().rearrange("(t p) m -> p t m", p=P)
    woT_r = woT.ap().rearrange("(h p) n -> p h n", p=P)

    with tile.TileContext(nc) as tc, ExitStack() as top:
        const = top.enter_context(tc.tile_pool(name="const", bufs=1))
        projstack = ExitStack()
        xp = projstack.enter_context(
            tc.tile_pool(name="xp", bufs=2, side="right"))
        wvp = projstack.enter_context(
            tc.tile_pool(name="wvp", bufs=1, side="right"))
        wqkp = projstack.enter_context(
            tc.tile_pool(name="wqkp", bufs=2, side="right"))
        wop = top.enter_context(tc.tile_pool(name="wop", bufs=1))
        kvp = top.enter_context(tc.tile_pool(name="kvp", bufs=1))
        qp = top.enter_context(tc.tile_pool(name="qp", bufs=2))
        ctxp = top.enter_context(tc.tile_pool(name="ctxp", bufs=2))
        ptp = top.enter_context(tc.tile_pool(name="ptp", bufs=5))
        csp = top.enter_context(tc.tile_pool(name="csp", bufs=2))
        denp = top.enter_context(tc.tile_pool(name="denp", bufs=2))
        evp = top.enter_context(tc.tile_pool(name="evp", bufs=2))
        # PSUM: psA = double-width score tiles (2 banks x 2 bufs), psB = attn
        # context accumulators, psG = shared projection/out-projection groups
        psA = top.enter_context(tc.tile_pool(name="psA", bufs=2, space="PSUM"))
        psB = top.enter_context(tc.tile_pool(name="psB", bufs=2, space="PSUM"))
        psG = top.enter_context(tc.tile_pool(name="psG", bufs=2, space="PSUM"))

        m0 = const.tile([P, P], BF16, tag="m0", name="m0")

        # persistent per-run SBUF state
        k_tiles = [[kvp.tile([P, QS], BF16, tag=f"k{h}_{c}", name=f"k{h}_{c}")
                    for c in range(NQ)] for h in range(HL)]
        v_tiles = [kvp.tile([P, ML], BF16, tag=f"v{st}", name=f"v{st}")
                   for st in range(NQ * 4)]
        q_map = [[None] * HL for _ in range(NQ)]
        ctx_map = [[None] * HL for _ in range(NQ)]

        wo_strips = [None] * 4
        ptp3 = [None]

        def load_wo():
            for nk in range(4):
                wo = wop.tile([P, HL, QS], BF16, tag=f"wo{nk}", name=f"wo{nk}")
                nc.sync.dma_start(
                    out=wo[:], in_=woT_r[:, :, nk * QS:(nk + 1) * QS])
                wo_strips[nk] = wo

        def outproj_gen(c, chunk):
            """Out-projection rows for the 4 s-tiles of quarter c."""
            cnt = 0
            for nk in range(4):
                wo = wo_strips[nk]
                for sti in range(4):
                    st = 4 * c + sti
                    ps = psG.tile([P, QS], F32, tag="pg", name=f"po{c}{nk}{sti}")
                    for h in range(HL):
                        nc.tensor.matmul(
                            ps[:],
                            ctx_map[c][h][:, sti * P:(sti + 1) * P],
                            wo[:, h, :],
                            start=(h == 0), stop=(h == HL - 1))
                        cnt += 1
                        if cnt % chunk == 0:
                            yield
                    ev = evp.tile([P, QS], BF16, tag="ev", name=f"ev{c}{nk}{sti}")
                    nc.scalar.copy(ev[:], ps[:])
                    nc.sync.dma_start(
                        out=out[st * P:(st + 1) * P, nk * QS:(nk + 1) * QS],
                        in_=ev[:])

        def attn_emit(h, c, slot):
            """Causal attention for (head h, query quarter c). `slot` is
            called ~once per k-tile to splice in interleaved matmuls.

            Off-diagonal k-tiles are processed in pairs sharing one
            double-width PSUM tile and a single exp over [128, 1024] (halves
            the Activation engine's fixed overhead); the P@V matmuls of each
            unit are emitted one unit late so the exp latency is hidden
            behind the next unit's score matmuls."""
            nj = 4 * (c + 1)
            ndiag = 4 * c
            qt = q_map[c][h]
            pctx = psB.tile([P, QS], F32, tag="pctx", name=f"pctx{h}_{c}")
            csum = csp.tile([P, QS], BF16, tag="csum", name=f"cs{h}_{c}")

            units = [("pair", J) for J in range(0, ndiag, 2)]
            units += [("diag", J) for J in range(ndiag, nj)]
            n_pv = 0
            pend = []

            def kslice(J):
                return k_tiles[h][J // 4][:, (J % 4) * P:(J % 4 + 1) * P]

            def flush():
                nonlocal n_pv
                for (pt_ap, J, r) in pend:
                    nc.tensor.matmul(
                        pctx[:, r:QS], v_tiles[J][:, h * P:(h + 1) * P],
                        pt_ap,
                        start=(n_pv == 0), stop=(n_pv == nj - 1))
                    n_pv += 1
                    if n_pv == 1:
                        nc.vector.tensor_copy(csum[:], pt_ap)
                    else:
                        nc.vector.tensor_add(
                            csum[:, r:QS], csum[:, r:QS], pt_ap)
                pend.clear()

            for kind, J in units:
                ps2 = psA.tile([P, 2 * QS], F32, tag="ps2",
                               name=f"sc{h}{c}{J}")
                if kind == "pair":
                    nc.tensor.matmul(ps2[:, 0:QS], kslice(J), qt[:],
                                     start=True, stop=True)
                    nc.tensor.matmul(ps2[:, QS:2 * QS], kslice(J + 1), qt[:],
                                     start=True, stop=True)
                    pool, nb = (ptp3[0], 4) if c == 3 else (ptp, 2)
                    pt2 = pool.tile([P, 2 * QS], BF16, tag="pt2", bufs=nb,
                                    name=f"pt{h}{c}{J}")
                    nc.scalar.activation(pt2[:], ps2[:], Exp, scale=ISQ)
                    slot()
                    flush()
                    slot()
                    pend.append((pt2[:, 0:QS], J, 0))
                    pend.append((pt2[:, QS:2 * QS], J + 1, 0))
                else:
                    r = J * P - c * QS
                    nc.tensor.matmul(ps2[:, r:QS], kslice(J), qt[:, r:QS],
                                     start=True, stop=True)
                    pool, nb = (ptp3[0], 4) if c == 3 else (ptp, 3)
                    pt = pool.tile([P, QS], BF16, tag="pt", bufs=nb,
                                   name=f"pt{h}{c}{J}")
                    nc.scalar.activation(pt[:, r:QS], ps2[:, r:QS], Exp,
                                         scale=ISQ)
                    nc.vector.tensor_mul(pt[:, r:r + P], pt[:, r:r + P], m0[:])
                    slot()
                    flush()
                    pend.append((pt[:, r:QS], J, r))
            flush()
            den = denp.tile([P, QS], BF16, tag="den", name=f"den{h}_{c}")
            nc.gpsimd.partition_all_reduce(
                den[:], csum[:], P, bass_isa.ReduceOp.add)
            ctx_t = ctxp.tile([P, QS], BF16, tag=f"ctx{h}", name=f"ctx{h}_{c}")
            nc.vector.tensor_tensor(ctx_t[:], pctx[:], den[:],
                                    mybir.AluOpType.divide)
            ctx_map[c][h] = ctx_t

        def run_all(gen):
            for _ in gen:
                pass

        def make_slot(gens):
            """Round-robin one step from each live generator per call."""
            live = list(gens)

            def slot():
                for g in list(live):
                    try:
                        next(g)
                    except StopIteration:
                        live.remove(g)
            return slot, live

        def proj_gen2(c, chunk):
            """QKV projections for quarter c; yields after every `chunk`
            matmuls so the caller can interleave them into the PE stream.
            q/k for heads 0-3 go between the two Wv halves so the
            single-buffered vr reload overlaps the qk matmuls."""
            cnt = 0
            xq = xp.tile([P, NDT, QS], BF16, tag="xq", name=f"xq{c}")
            for i in range(4):
                nc.sync.dma_start(
                    out=xq[:, 4 * i:4 * i + 4, :],
                    in_=xT_r[:, 4 * i:4 * i + 4, c * QS:(c + 1) * QS])
            for mh in range(2):
                vr = wvp.tile([P, NDT, QS], BF16, tag="vr", name=f"vr{c}_{mh}")
                for i in range(4):
                    nc.sync.dma_start(
                        out=vr[:, 4 * i:4 * i + 4, :],
                        in_=wvT_r[:, 4 * i:4 * i + 4, mh * QS:(mh + 1) * QS])
                for sti in range(4):
                    st = 4 * c + sti
                    ps = psG.tile([P, QS], F32, tag="pg", name=f"pv{c}{mh}{sti}")
                    for d in range(NDT):
                        nc.tensor.matmul(
                            ps[:], xq[:, d, sti * P:(sti + 1) * P], vr[:, d, :],
                            start=(d == 0), stop=(d == NDT - 1))
                        cnt += 1
                        if cnt % chunk == 0:
                            yield
                    nc.vector.tensor_copy(
                        v_tiles[st][:, mh * QS:(mh + 1) * QS], ps[:])
                heads = range(4) if mh == 0 else range(4, HL)
                for h in heads:
                    wqk = wqkp.tile([P, NDT, 2 * P], BF16, tag="wqk",
                                    name=f"wqk{c}_{h}")
                    for i in range(2):
                        nc.sync.dma_start(
                            out=wqk[:, 8 * i:8 * i + 8, :],
                            in_=wqkT_r[:, 8 * i:8 * i + 8,
                                       h * 2 * P:(h + 1) * 2 * P])
                    qt = qp.tile([P, QS], BF16, tag=f"q{h}", name=f"q{h}_{c}")
                    q_map[c][h] = qt
                    for (off, dst) in ((0, qt), (P, k_tiles[h][c])):
                        ps = psG.tile([P, QS], F32, tag="pg",
                                      name=f"pq{c}{h}{off}")
                        for d in range(NDT):
                            nc.tensor.matmul(
                                ps[:], wqk[:, d, off:off + P], xq[:, d, :],
                                start=(d == 0), stop=(d == NDT - 1))
                            cnt += 1
                            if cnt % chunk == 0:
                                yield
                        nc.vector.tensor_copy(dst[:], ps[:])

        def prologue():
            """Quarter-0 projections with DMA piece interleave and, for the
            V part, 4 concurrently-open PSUM groups iterated d-piece-outer,
            so the PE starts as soon as the first 0.5MB pieces land and the
            DMA stream stays ahead of the 4x-slower PE consumption."""
            PIECES = [(0, 1), (1, 2), (2, 4), (4, 6), (6, 8), (8, 12),
                      (12, 16)]
            xq = xp.tile([P, NDT, QS], BF16, tag="xq", name="xq0")
            vr0 = wvp.tile([P, NDT, QS], BF16, tag="vr", name="vr0_0")
            for (d0, d1) in PIECES:
                nc.sync.dma_start(out=xq[:, d0:d1, :],
                                  in_=xT_r[:, d0:d1, 0:QS])
                nc.sync.dma_start(out=vr0[:, d0:d1, :],
                                  in_=wvT_r[:, d0:d1, 0:QS])

            def v_half(mh, vr):
                psa = psA.tile([P, 2 * QS], F32, tag="ps2", name=f"pv0{mh}ab")
                ps = [psG.tile([P, QS], F32, tag="pg", name=f"pv0{mh}0"),
                      psG.tile([P, QS], F32, tag="pg", name=f"pv0{mh}1"),
                      psa[:, 0:QS], psa[:, QS:2 * QS]]
                for (d0, d1) in PIECES:
                    for g in range(4):
                        for d in range(d0, d1):
                            nc.tensor.matmul(
                                ps[g][:], xq[:, d, g * P:(g + 1) * P],
                                vr[:, d, :],
                                start=(d == 0), stop=(d == NDT - 1))
                for g in range(4):
                    nc.vector.tensor_copy(
                        v_tiles[g][:, mh * QS:(mh + 1) * QS], ps[g][:])

            def qk_heads(heads):
                for h in heads:
                    wqk = wqkp.tile([P, NDT, 2 * P], BF16, tag="wqk",
                                    name=f"wqk0_{h}")
                    for i in range(2):
                        nc.sync.dma_start(
                            out=wqk[:, 8 * i:8 * i + 8, :],
                            in_=wqkT_r[:, 8 * i:8 * i + 8,
                                       h * 2 * P:(h + 1) * 2 * P])
                    qt = qp.tile([P, QS], BF16, tag=f"q{h}", name=f"q{h}_0")
                    q_map[0][h] = qt
                    for (off, dst) in ((0, qt), (P, k_tiles[h][0])):
                        ps = psG.tile([P, QS], F32, tag="pg",
                                      name=f"pq0{h}{off}")
                        for d in range(NDT):
                            nc.tensor.matmul(
                                ps[:], wqk[:, d, off:off + P], xq[:, d, :],
                                start=(d == 0), stop=(d == NDT - 1))
                        nc.vector.tensor_copy(dst[:], ps[:])

            v_half(0, vr0)
            qk_heads(range(4))
            load_wo()
            nc.sync.dma_start(out=m0[:], in_=mask0[:])
            vr1 = wvp.tile([P, NDT, QS], BF16, tag="vr", name="vr0_1")
            for i in range(4):
                nc.sync.dma_start(out=vr1[:, 4 * i:4 * i + 4, :],
                                  in_=wvT_r[:, 4 * i:4 * i + 4, QS:2 * QS])
            v_half(1, vr1)
            qk_heads(range(4, HL))

        prologue()

        # chunk sizes per cycle chosen so each stream exhausts just before
        # its cycle's attention slots do (slots = 8 * nj(c))
        proj_chunk = {0: 12, 1: 6, 2: 4}
        op_chunk = {1: 2, 2: 2, 3: 1}
        # cycle-3 interleave weights per head (16 slot calls: 6 pairs x 2 +
        # 4 diag): front-loaded so the first pair-flush never waits its exp,
        # and 8 of outproj(2)'s 128 matmuls are held past the head loop to
        # cover the last head's denominator-chain latency before outproj(3)
        W3 = [2, 1, 1, 1, 1, 0, 1, 0, 1, 0, 1, 0, 1, 1, 1, 1]
        for c in range(NQ):
            gens = []
            if c < 3:
                gens.append(proj_gen2(c + 1, proj_chunk[c]))
            if c >= 1:
                gens.append(outproj_gen(c - 1, op_chunk[c]))
            slot, live = make_slot(gens)
            if c == 3:
                # all projections are done: reclaim the x/wv/wqk streaming
                # space for a deeper probability-tile pool
                projstack.close()
                ptp3[0] = top.enter_context(
                    tc.tile_pool(name="ptp3", bufs=4, side="right"))
                base, calls = slot, [0]

                def slot():
                    w = W3[calls[0] % 16]
                    calls[0] += 1
                    for _ in range(w):
                        base()
            for h in range(HL):
                attn_emit(h, c, slot)
            for g in live:
                run_all(g)
        run_all(outproj_gen(3, 10 ** 9))

    nc.finalize()
    return nc


def get_nc():
    if "nc" not in _BUILT:
        _BUILT["nc"] = _build()
    return _BUILT["nc"]


def _bf16(a):
    import ml_dtypes
    return np.ascontiguousarray(a, dtype=np.float32).astype(ml_dtypes.bfloat16)


def _make_in_maps(x, Wq, Wk, Wv, Wo):
    jj, ff = np.meshgrid(np.arange(P), np.arange(P), indexing="ij")
    mask0 = (ff >= jj).astype(np.float32)
    in_maps = []
    for c in range(8):
        b, t = c // 2, c % 2
        ms = slice(t * ML, (t + 1) * ML)
        wq = Wq[ms, :].T  # [D, ML]
        wk = Wk[ms, :].T
        # interleave per head: [q_h (128 cols) | k_h (128 cols)] blocks
        wqk = np.empty((D, 2 * ML), dtype=np.float32)
        for h in range(HL):
            wqk[:, 2 * h * P:(2 * h + 1) * P] = wq[:, h * P:(h + 1) * P]
            wqk[:, (2 * h + 1) * P:(2 * h + 2) * P] = wk[:, h * P:(h + 1) * P]
        in_maps.append({
            "xT": _bf16(x[b].T),
            "wqkT": _bf16(wqk),
            "wvT": _bf16(Wv[ms, :].T),
            "woT": _bf16(Wo[:, ms].T),
            "mask0": _bf16(mask0),
        })
    return in_maps


def kernel(x, Wq, Wk, Wv, Wo):
    x = np.asarray(x, dtype=np.float32)
    Wq = np.asarray(Wq, dtype=np.float32)
    Wk = np.asarray(Wk, dtype=np.float32)
    Wv = np.asarray(Wv, dtype=np.float32)
    Wo = np.asarray(Wo, dtype=np.float32)

    nc = get_nc()
    in_maps = _make_in_maps(x, Wq, Wk, Wv, Wo)
    res = run_bass_kernel_spmd(nc, in_maps, list(range(8)))
    outs = [np.asarray(res.results[c]["out"]).astype(np.float32)
            for c in range(8)]
    full = np.stack([outs[2 * b] + outs[2 * b + 1] for b in range(B)])
    return full.astype(np.float32)
